# revision 1
# baseline (speedup 1.0000x reference)
"""TRN2 Bass kernel for nn_Block_72464688218281 (gnn_message_passing).

Reference computation, per batch b, point n, neighbor k (g = neigh_idx[b,n,k]):
    dist = |xyz_n - xyz_g|
    f10  = [dist, xyz_n - xyz_g, xyz_n, xyz_g]
    y[b,:,n,k] = relu(BN(W @ f10))
which folds algebraically (scale = gamma/sqrt(var+eps)) to
    y_o = relu(W0'_o*dist + A'_o.xyz_n + Bc'_o.xyz_g + shift_o)
with W0' = scale*W[:,0], A' = scale*(W[:,4:7]+W[:,1:4]),
Bc' = scale*(W[:,7:10]-W[:,1:4]), shift = beta - mean*scale.

Distribution: shard the point dim N across the 8 cores (each core handles
N/8 = 5120 points of every batch; gathers via neigh_idx are intra-sample so
each core only needs the full per-point table, which is replicated).

Device pipeline per (batch, supertile of 1024 points):
  - indirect-DMA gather of per-pair point records [xyz, v=Bc'@xyz] from a
    DRAM table (one record per (point, neighbor) pair),
  - dist via DVE (rel, square, sum) + ACT sqrt,
  - o-expansion y = dist*W0' + (v + U) elementwise on DVE, where
    U[o, n] = A'_o.xyz_n + shift_o is built once per batch on DVE,
  - relu + (i,o)->(o,i) plane transpose on ACT,
  - 64KB-contiguous o-plane stores.

Layout: within a supertile, partition p owns points [8p, 8p+8); free slot
i = (pt*16 + k). Output plane (b, o, supertile) is a single contiguous
64KB DRAM block.
"""
import sys
import types

import numpy as np

sys.path.insert(0, "/opt/trn_rl_repo")

B, N, K = 4, 40960, 16
DO = 16
EPS = 1e-5
NCORES = 8
SH = N // NCORES          # 5120 points per core per batch
ST = 1024                 # points per supertile
NST = SH // ST            # 5 supertiles per batch per core
PPT = ST // 128           # 8 points per partition per supertile
MO = PPT * K              # 128 pair slots per partition per supertile
REC = 20                  # f32 per record: [x, y, z, v0..15, pad]

_CACHE = {}


def _install_ntff_hook():
    """The container's antenv stub lacks axon_hooks; install it so
    run_bass_kernel_spmd(trace=True) can capture NTFF profiles."""
    if "antenv.axon_hooks" in sys.modules:
        return
    try:
        import antenv
        from trn_agent_boot.trn_boot import _ntff_profile_via_ctypes
    except Exception:
        return
    mod = types.ModuleType("antenv.axon_hooks")
    state = {"hook": None}
    mod.set_axon_ntff_profile_hook = lambda h: state.__setitem__("hook", h)
    mod.get_axon_ntff_profile_hook = lambda: state["hook"]
    sys.modules["antenv.axon_hooks"] = mod
    antenv.axon_hooks = mod
    try:
        mod.set_axon_ntff_profile_hook(
            _ntff_profile_via_ctypes("/opt/axon/libaxon_pjrt.so")
        )
    except Exception:
        pass


def _build_program():
    import concourse.bass as bass
    import concourse.bacc as bacc
    import concourse.mybir as mybir
    import concourse.tile as tile

    P = 128
    f32 = mybir.dt.float32
    i32 = mybir.dt.int32
    mult = mybir.AluOpType.mult
    add = mybir.AluOpType.add
    sub = mybir.AluOpType.subtract

    nc = bacc.Bacc("TRN2", target_bir_lowering=False, debug=False,
                   num_devices=NCORES)

    tbl = nc.dram_tensor("tbl", [B * N, REC], f32, kind="ExternalInput")
    offs = nc.dram_tensor("offs", [B * NST, P, MO], i32, kind="ExternalInput")
    ctr = nc.dram_tensor("ctr", [B, P, NST * 24], f32, kind="ExternalInput")
    arow = nc.dram_tensor("arow", [128, 80], f32, kind="ExternalInput")
    yout = nc.dram_tensor("yout", [B, DO, NST, P * MO], f32,
                          kind="ExternalOutput")

    with tile.TileContext(nc) as tc:
        with (
            tc.tile_pool(name="cst", bufs=1) as cst,
            tc.tile_pool(name="offp", bufs=3) as offp,
            tc.tile_pool(name="gp", bufs=3) as gp,
            tc.tile_pool(name="wku", bufs=2) as wku,
            tc.tile_pool(name="wk", bufs=2) as wk,
            tc.tile_pool(name="yp", bufs=3) as yp,
        ):
            at = cst.tile([128, 80], f32)
            nc.sync.dma_start(out=at[:], in_=arow[:])
            ctrt = cst.tile([P, B * NST * 24], f32)
            for b in range(B):
                nc.sync.dma_start(
                    out=ctrt[:, b * NST * 24 : (b + 1) * NST * 24],
                    in_=ctr[b, :, :],
                )

            for b in range(B):
                cs = ctrt[:, b * NST * 24 : (b + 1) * NST * 24]
                # cs layout per partition: (st, c, pt) -> st*24 + c*8 + pt
                c4 = cs.rearrange("p (s c t) -> p s c t", c=3, t=8)

                # U[p, (s t o)] = sum_c A'[o,c]*ctr_c + shift_o
                U = wku.tile([P, NST * PPT * DO], f32, tag="U")
                U4 = U[:].rearrange("p (s t o) -> p s t o", s=NST, t=PPT)
                m0 = wku.tile([P, NST * PPT * DO], f32, tag="m0")
                m04 = m0[:].rearrange("p (s t o) -> p s t o", s=NST, t=PPT)

                def cexp(c):
                    # [128, s, t] -> append step-0 o dim
                    return c4[:, :, c, :].to_broadcast([P, NST, PPT, DO])

                def abc(lo):
                    # arow[lo:lo+16] broadcast over (s, t)
                    a = at[:, lo : lo + 16].to_broadcast([P, 16, NST, PPT])
                    return a.rearrange("p o s t -> p s t o")

                nc.vector.tensor_tensor(out=U4, in0=cexp(0), in1=abc(0),
                                        op=mult)
                nc.vector.tensor_tensor(out=m04, in0=cexp(1), in1=abc(16),
                                        op=mult)
                nc.vector.tensor_tensor(out=U[:], in0=U[:], in1=m0[:], op=add)
                nc.vector.tensor_tensor(out=m04, in0=cexp(2), in1=abc(32),
                                        op=mult)
                nc.vector.tensor_tensor(out=U[:], in0=U[:], in1=m0[:], op=add)
                nc.vector.tensor_tensor(out=U4, in0=U4, in1=abc(48), op=add)

                for st in range(NST):
                    bs = b * NST + st
                    ot = offp.tile([P, MO], i32, tag="ot")
                    nc.sync.dma_start(out=ot[:], in_=offs[bs, :, :])
                    G = gp.tile([P, MO * REC], f32, tag="G")
                    for c in range(MO):
                        nc.gpsimd.indirect_dma_start(
                            out=G[:, c * REC : (c + 1) * REC],
                            out_offset=None,
                            in_=tbl[:],
                            in_offset=bass.IndirectOffsetOnAxis(
                                ap=ot[:, c : c + 1], axis=0
                            ),
                        )
                    g3 = G[:].rearrange("p (i e) -> p i e", e=REC)

                    # rel (c-major slices), sq, d2, dist
                    rel = wk.tile([P, 3 * MO], f32, tag="rel")
                    for c in range(3):
                        nc.vector.tensor_tensor(
                            out=rel[:, c * MO : (c + 1) * MO]
                            .rearrange("p (t k) -> p t k", t=PPT),
                            in0=g3[:, :, c]
                            .rearrange("p (t k) -> p t k", t=PPT),
                            in1=c4[:, st, c, :].to_broadcast([P, PPT, K]),
                            op=sub,
                        )
                    sq = wk.tile([P, 3 * MO], f32, tag="sq")
                    nc.vector.tensor_tensor(out=sq[:], in0=rel[:], in1=rel[:],
                                            op=mult)
                    d2 = wk.tile([P, MO], f32, tag="d2")
                    nc.vector.tensor_tensor(
                        out=d2[:], in0=sq[:, 0:MO], in1=sq[:, MO : 2 * MO],
                        op=add,
                    )
                    nc.vector.tensor_tensor(
                        out=d2[:], in0=d2[:], in1=sq[:, 2 * MO : 3 * MO],
                        op=add,
                    )
                    dist = wk.tile([P, MO], f32, tag="dist")
                    nc.scalar.activation(
                        dist[:], d2[:], mybir.ActivationFunctionType.Sqrt
                    )

                    # acc = v + U ;  yv = dist*W0' + acc   (order (t, k, o))
                    acc = wk.tile([P, MO * DO], f32, tag="acc")
                    acc4 = acc[:].rearrange("p (t k o) -> p t k o", t=PPT,
                                            k=K)
                    nc.vector.tensor_tensor(
                        out=acc4,
                        in0=g3[:, :, 3 : 3 + DO]
                        .rearrange("p (t k) o -> p t k o", t=PPT),
                        in1=U4[:, st].to_broadcast([P, PPT, DO, K])
                        .rearrange("p t o k -> p t k o"),
                        op=add,
                    )
                    yv = wk.tile([P, MO * DO], f32, tag="yv")
                    yv4 = yv[:].rearrange("p (t k o) -> p t k o", t=PPT, k=K)
                    nc.vector.tensor_tensor(
                        out=yv4,
                        in0=dist[:].rearrange("p (t k) -> p t k", t=PPT)
                        .to_broadcast([P, PPT, K, DO]),
                        in1=at[:, 64:80]
                        .to_broadcast([P, 16, PPT, K])
                        .rearrange("p o t k -> p t k o"),
                        op=mult,
                    )
                    nc.vector.tensor_tensor(out=yv[:], in0=yv[:], in1=acc[:],
                                            op=add)

                    # relu + (t,k,o) -> (o, t, k) plane layout
                    yplan = yp.tile([P, DO * MO], f32, tag="yplan")
                    nc.scalar.activation(
                        yplan[:].rearrange("p (o t k) -> p o t k", t=PPT,
                                           k=K)
                        .rearrange("p o t k -> p t k o"),
                        yv4,
                        mybir.ActivationFunctionType.Relu,
                    )
                    for o in range(DO):
                        nc.sync.dma_start(
                            out=yout[b, o, st, :].rearrange("(p i) -> p i",
                                                            p=P),
                            in_=yplan[:, o * MO : (o + 1) * MO],
                        )
    nc.compile()
    return nc


def _prepare_inputs(xyz, neigh_idx, W, gamma, beta, mean, var):
    scale = gamma / np.sqrt(var + EPS)
    W0p = scale * W[:, 0]
    Ap = scale[:, None] * (W[:, 4:7] + W[:, 1:4])
    Bcp = scale[:, None] * (W[:, 7:10] - W[:, 1:4])
    shiftp = beta - mean * scale

    T = np.zeros((B, N, REC), np.float32)
    T[:, :, 0:3] = xyz
    T[:, :, 3:19] = xyz @ Bcp.T
    T = np.ascontiguousarray(T.reshape(B * N, REC))

    arow1 = np.zeros((1, 80), np.float32)
    arow1[0, 0:16] = Ap[:, 0]
    arow1[0, 16:32] = Ap[:, 1]
    arow1[0, 32:48] = Ap[:, 2]
    arow1[0, 48:64] = shiftp
    arow1[0, 64:80] = W0p
    arow = np.repeat(arow1, 128, axis=0)

    idx = neigh_idx.astype(np.int64)
    in_maps = []
    for c in range(NCORES):
        n0 = c * SH
        sl = idx[:, n0 : n0 + SH, :]  # [B, SH, K]
        # slot (b, st, p, i=(pt*16+k)) <- point n0 + st*1024 + p*8 + pt
        off = (
            sl.reshape(B, NST, 128, PPT, K)
            + (np.arange(B, dtype=np.int64) * N)[:, None, None, None, None]
        ).reshape(B * NST, 128, MO).astype(np.int32)
        xs = xyz[:, n0 : n0 + SH, :]  # [B, SH, 3]
        # ctr[b, p, st*24 + c*8 + pt]
        ctr = np.ascontiguousarray(
            xs.reshape(B, NST, 128, PPT, 3).transpose(0, 2, 1, 4, 3)
        ).reshape(B, 128, NST * 24)
        in_maps.append(
            {
                "tbl": T,
                "offs": np.ascontiguousarray(off),
                "ctr": np.ascontiguousarray(ctr.astype(np.float32)),
                "arow": arow,
            }
        )
    return in_maps


def kernel(xyz, feature, neigh_idx, W, gamma, beta, running_mean,
           running_var, _want_trace=False):
    _install_ntff_hook()
    from concourse import bass_utils

    xyz = np.asarray(xyz, np.float32)
    W = np.asarray(W, np.float32)
    gamma = np.asarray(gamma, np.float32)
    beta = np.asarray(beta, np.float32)
    mean = np.asarray(running_mean, np.float32)
    var = np.asarray(running_var, np.float32)

    if "prog" not in _CACHE:
        _CACHE["prog"] = _build_program()
    nc = _CACHE["prog"]

    in_maps = _prepare_inputs(xyz, np.asarray(neigh_idx), W, gamma, beta,
                              mean, var)
    res = bass_utils.run_bass_kernel_spmd(
        nc, in_maps, core_ids=list(range(NCORES)), trace=_want_trace
    )
    out = np.zeros((B, DO, N, K), np.float32)
    for c in range(NCORES):
        yc = res.results[c]["yout"].reshape(B, DO, SH, K)
        out[:, :, c * SH : (c + 1) * SH, :] = yc
    if _want_trace:
        return out, res.exec_time_ns
    return out



# revision 7
# speedup vs baseline: 19.4844x; 19.4844x over previous
"""TRN2 Bass kernel for nn_Block_72464688218281 (gnn_message_passing).

Reference computation, per batch b, point n, neighbor k (g = neigh_idx[b,n,k]):
    dist = |xyz_n - xyz_g|
    f10  = [dist, xyz_n - xyz_g, xyz_n, xyz_g]
    y[b,:,n,k] = relu(BN(W @ f10))
which folds algebraically (scale = gamma/sqrt(var+eps)) to
    y_o = relu(W0'_o*dist + A'_o.xyz_n + Bc'_o.xyz_g + shift_o)
with W0' = scale*W[:,0], A' = scale*(W[:,4:7]+W[:,1:4]),
Bc' = scale*(W[:,7:10]-W[:,1:4]), shift = beta - mean*scale.

Distribution: shard the point dim N across the 8 cores (each core handles
N/8 = 5120 points of every batch; gathers via neigh_idx are intra-sample so
each core only needs the full per-point table, which is replicated).

Device pipeline per (batch, supertile of 1024 points):
  - indirect-DMA gather of per-pair point records [xyz, v=Bc'@xyz] from a
    DRAM table (one record per (point, neighbor) pair),
  - dist via DVE (rel, square, sum) + ACT sqrt,
  - o-expansion y = dist*W0' + (v + U) elementwise on DVE, where
    U[o, n] = A'_o.xyz_n + shift_o is built once per batch on DVE,
  - relu + (i,o)->(o,i) plane transpose on ACT,
  - 64KB-contiguous o-plane stores.

Layout: within a supertile, partition p owns points [8p, 8p+8); free slot
i = (pt*16 + k). Output plane (b, o, supertile) is a single contiguous
64KB DRAM block.
"""
import sys
import types

import numpy as np

sys.path.insert(0, "/opt/trn_rl_repo")

B, N, K = 4, 40960, 16
DO = 16
EPS = 1e-5
NCORES = 8
SH = N // NCORES          # 5120 points per core per batch
ST = 1024                 # points per supertile
NST = SH // ST            # 5 supertiles per batch per core
PPT = ST // 128           # 8 points per partition per supertile
MO = PPT * K              # 128 pair slots per partition per supertile
REC = 20                  # f32 per record: [x, y, z, v0..15, pad]

_CACHE = {}


def _install_ntff_hook():
    """The container's antenv stub lacks axon_hooks; install it so
    run_bass_kernel_spmd(trace=True) can capture NTFF profiles."""
    if "antenv.axon_hooks" in sys.modules:
        return
    try:
        import antenv
        from trn_agent_boot.trn_boot import _ntff_profile_via_ctypes
    except Exception:
        return
    mod = types.ModuleType("antenv.axon_hooks")
    state = {"hook": None}
    mod.set_axon_ntff_profile_hook = lambda h: state.__setitem__("hook", h)
    mod.get_axon_ntff_profile_hook = lambda: state["hook"]
    sys.modules["antenv.axon_hooks"] = mod
    antenv.axon_hooks = mod
    try:
        mod.set_axon_ntff_profile_hook(
            _ntff_profile_via_ctypes("/opt/axon/libaxon_pjrt.so")
        )
    except Exception:
        pass


def _build_program():
    import concourse.bass as bass
    import concourse.bacc as bacc
    import concourse.mybir as mybir
    import concourse.tile as tile

    P = 128
    f32 = mybir.dt.float32
    i32 = mybir.dt.int32
    mult = mybir.AluOpType.mult
    add = mybir.AluOpType.add
    sub = mybir.AluOpType.subtract

    nc = bacc.Bacc("TRN2", target_bir_lowering=False, debug=False,
                   num_devices=NCORES)

    tbl = nc.dram_tensor("tbl", [B * N, REC], f32, kind="ExternalInput")
    offs = nc.dram_tensor("offs", [B * NST, P, MO], i32, kind="ExternalInput")
    ctr = nc.dram_tensor("ctr", [B, P, NST * 24], f32, kind="ExternalInput")
    arow = nc.dram_tensor("arow", [128, 80], f32, kind="ExternalInput")
    yout = nc.dram_tensor("yout", [B, NST, DO, P * MO], f32,
                          kind="ExternalOutput")

    with tile.TileContext(nc) as tc:
        with (
            tc.tile_pool(name="cst", bufs=1) as cst,
            tc.tile_pool(name="offp", bufs=3) as offp,
            tc.tile_pool(name="gp", bufs=3) as gp,
            tc.tile_pool(name="wku", bufs=2) as wku,
            tc.tile_pool(name="wk", bufs=2) as wk,
            tc.tile_pool(name="yp", bufs=3) as yp,
        ):
            at = cst.tile([128, 80], f32)
            nc.sync.dma_start(out=at[:], in_=arow[:])
            ctrt = cst.tile([P, B * NST * 24], f32)
            for b in range(B):
                nc.sync.dma_start(
                    out=ctrt[:, b * NST * 24 : (b + 1) * NST * 24],
                    in_=ctr[b, :, :],
                )

            for b in range(B):
                cs = ctrt[:, b * NST * 24 : (b + 1) * NST * 24]
                # cs layout per partition: (st, c, pt) -> st*24 + c*8 + pt
                c4 = cs.rearrange("p (s c t) -> p s c t", c=3, t=8)

                # U[p, (s t o)] = sum_c A'[o,c]*ctr_c + shift_o
                U = wku.tile([P, NST * PPT * DO], f32, tag="U")
                U4 = U[:].rearrange("p (s t o) -> p s t o", s=NST, t=PPT)
                m0 = wku.tile([P, NST * PPT * DO], f32, tag="m0")
                m04 = m0[:].rearrange("p (s t o) -> p s t o", s=NST, t=PPT)

                def cexp(c):
                    # [128, s, t] -> append step-0 o dim
                    return c4[:, :, c, :].to_broadcast([P, NST, PPT, DO])

                def abc(lo):
                    # arow[lo:lo+16] broadcast over (s, t)
                    a = at[:, lo : lo + 16].to_broadcast([P, 16, NST, PPT])
                    return a.rearrange("p o s t -> p s t o")

                nc.vector.tensor_tensor(out=U4, in0=cexp(0), in1=abc(0),
                                        op=mult)
                nc.vector.tensor_tensor(out=m04, in0=cexp(1), in1=abc(16),
                                        op=mult)
                nc.vector.tensor_tensor(out=U[:], in0=U[:], in1=m0[:], op=add)
                nc.vector.tensor_tensor(out=m04, in0=cexp(2), in1=abc(32),
                                        op=mult)
                nc.vector.tensor_tensor(out=U[:], in0=U[:], in1=m0[:], op=add)
                nc.vector.tensor_tensor(out=U4, in0=U4, in1=abc(48), op=add)

                for st in range(NST):
                    bs = b * NST + st
                    ot = offp.tile([P, MO], i32, tag="ot")
                    nc.sync.dma_start(out=ot[:], in_=offs[bs, :, :])
                    G = gp.tile([P, MO * REC], f32, tag="G")
                    for c in range(MO):
                        nc.gpsimd.indirect_dma_start(
                            out=G[:, c * REC : (c + 1) * REC],
                            out_offset=None,
                            in_=tbl[:],
                            in_offset=bass.IndirectOffsetOnAxis(
                                ap=ot[:, c : c + 1], axis=0
                            ),
                        )
                    g3 = G[:].rearrange("p (i e) -> p i e", e=REC)

                    # rel (c-major slices), sq, d2, dist
                    rel = wk.tile([P, 3 * MO], f32, tag="rel")
                    for c in range(3):
                        nc.vector.tensor_tensor(
                            out=rel[:, c * MO : (c + 1) * MO]
                            .rearrange("p (t k) -> p t k", t=PPT),
                            in0=g3[:, :, c]
                            .rearrange("p (t k) -> p t k", t=PPT),
                            in1=c4[:, st, c, :].to_broadcast([P, PPT, K]),
                            op=sub,
                        )
                    sq = wk.tile([P, 3 * MO], f32, tag="sq")
                    nc.vector.tensor_tensor(out=sq[:], in0=rel[:], in1=rel[:],
                                            op=mult)
                    d2 = wk.tile([P, MO], f32, tag="d2")
                    nc.vector.tensor_tensor(
                        out=d2[:], in0=sq[:, 0:MO], in1=sq[:, MO : 2 * MO],
                        op=add,
                    )
                    nc.vector.tensor_tensor(
                        out=d2[:], in0=d2[:], in1=sq[:, 2 * MO : 3 * MO],
                        op=add,
                    )
                    dist = wk.tile([P, MO], f32, tag="dist")
                    nc.scalar.activation(
                        dist[:], d2[:], mybir.ActivationFunctionType.Sqrt
                    )

                    # acc = v + U ;  yv = dist*W0' + acc   (order (t, k, o))
                    acc = wk.tile([P, MO * DO], f32, tag="acc")
                    acc4 = acc[:].rearrange("p (t k o) -> p t k o", t=PPT,
                                            k=K)
                    nc.vector.tensor_tensor(
                        out=acc4,
                        in0=g3[:, :, 3 : 3 + DO]
                        .rearrange("p (t k) o -> p t k o", t=PPT),
                        in1=U4[:, st].to_broadcast([P, PPT, DO, K])
                        .rearrange("p t o k -> p t k o"),
                        op=add,
                    )
                    yv = wk.tile([P, MO * DO], f32, tag="yv")
                    yv4 = yv[:].rearrange("p (t k o) -> p t k o", t=PPT, k=K)
                    nc.vector.tensor_tensor(
                        out=yv4,
                        in0=dist[:].rearrange("p (t k) -> p t k", t=PPT)
                        .to_broadcast([P, PPT, K, DO]),
                        in1=at[:, 64:80]
                        .to_broadcast([P, 16, PPT, K])
                        .rearrange("p o t k -> p t k o"),
                        op=mult,
                    )
                    nc.vector.tensor_tensor(out=yv[:], in0=yv[:], in1=acc[:],
                                            op=add)

                    # relu + (t,k,o) -> (o, t, k) plane layout
                    yplan = yp.tile([P, DO * MO], f32, tag="yplan")
                    nc.scalar.activation(
                        yplan[:].rearrange("p (o t k) -> p o t k", t=PPT,
                                           k=K)
                        .rearrange("p o t k -> p t k o"),
                        yv4,
                        mybir.ActivationFunctionType.Relu,
                    )
                    nc.sync.dma_start(
                        out=yout[b, st, :, :]
                        .rearrange("o (p i) -> p o i", p=P),
                        in_=yplan[:].rearrange("p (o i) -> p o i", o=DO),
                    )
    nc.compile()
    return nc


def _prepare_inputs(xyz, neigh_idx, W, gamma, beta, mean, var):
    scale = gamma / np.sqrt(var + EPS)
    W0p = scale * W[:, 0]
    Ap = scale[:, None] * (W[:, 4:7] + W[:, 1:4])
    Bcp = scale[:, None] * (W[:, 7:10] - W[:, 1:4])
    shiftp = beta - mean * scale

    T = np.zeros((B, N, REC), np.float32)
    T[:, :, 0:3] = xyz
    T[:, :, 3:19] = xyz @ Bcp.T
    T = np.ascontiguousarray(T.reshape(B * N, REC))

    arow1 = np.zeros((1, 80), np.float32)
    arow1[0, 0:16] = Ap[:, 0]
    arow1[0, 16:32] = Ap[:, 1]
    arow1[0, 32:48] = Ap[:, 2]
    arow1[0, 48:64] = shiftp
    arow1[0, 64:80] = W0p
    arow = np.repeat(arow1, 128, axis=0)

    idx = neigh_idx.astype(np.int64)
    in_maps = []
    for c in range(NCORES):
        n0 = c * SH
        sl = idx[:, n0 : n0 + SH, :]  # [B, SH, K]
        # slot (b, st, p, i=(pt*16+k)) <- point n0 + st*1024 + p*8 + pt
        off = (
            sl.reshape(B, NST, 128, PPT, K)
            + (np.arange(B, dtype=np.int64) * N)[:, None, None, None, None]
        ).reshape(B * NST, 128, MO).astype(np.int32)
        xs = xyz[:, n0 : n0 + SH, :]  # [B, SH, 3]
        # ctr[b, p, st*24 + c*8 + pt]
        ctr = np.ascontiguousarray(
            xs.reshape(B, NST, 128, PPT, 3).transpose(0, 2, 1, 4, 3)
        ).reshape(B, 128, NST * 24)
        in_maps.append(
            {
                "tbl": T,
                "offs": np.ascontiguousarray(off),
                "ctr": np.ascontiguousarray(ctr.astype(np.float32)),
                "arow": arow,
            }
        )
    return in_maps


def kernel(xyz, feature, neigh_idx, W, gamma, beta, running_mean,
           running_var, _want_trace=False):
    _install_ntff_hook()
    from concourse import bass_utils

    xyz = np.asarray(xyz, np.float32)
    W = np.asarray(W, np.float32)
    gamma = np.asarray(gamma, np.float32)
    beta = np.asarray(beta, np.float32)
    mean = np.asarray(running_mean, np.float32)
    var = np.asarray(running_var, np.float32)

    if "prog" not in _CACHE:
        _CACHE["prog"] = _build_program()
    nc = _CACHE["prog"]

    in_maps = _prepare_inputs(xyz, np.asarray(neigh_idx), W, gamma, beta,
                              mean, var)
    res = bass_utils.run_bass_kernel_spmd(
        nc, in_maps, core_ids=list(range(NCORES)), trace=_want_trace
    )
    out = np.zeros((B, DO, N, K), np.float32)
    for c in range(NCORES):
        yc = (
            res.results[c]["yout"]
            .reshape(B, NST, DO, ST, K)
            .transpose(0, 2, 1, 3, 4)
            .reshape(B, DO, SH, K)
        )
        out[:, :, c * SH : (c + 1) * SH, :] = yc
    if _want_trace:
        return out, res.exec_time_ns
    return out

